# revision 22
# baseline (speedup 1.0000x reference)
"""SS2D (VMamba-style 2D selective scan) Trainium2 Bass kernel.

Full inputs in, full output out. Internally: 8-core SPMD.

Sharding: core c = (b, dh, nh):  b = c//4, dh = (c//2)%2, nh = c%2.
  - b: batch.
  - dh: d_inner half (96 channels of 192).
  - nh: d_state half (8 of 16 states), processed for BOTH scan directions k.

v3 structure:
  - Everything after the conv stays in DIAGONAL order; the host un-permutes
    the output (and pre-gathers the z-gate input), so the on-device un-diag
    reorder pass is gone.
  - The scan loop runs direction k=1 first, evicts its partial y, and ships
    it through a ReduceScatter that overlaps the whole k=0 compute phase;
    only the k=0 ReduceScatter is exposed.
  - B/C broadcast DMAs are issued from two different queues (sync/scalar)
    with deeper buffering and no in-place aliasing, so they pipeline with
    the scan instead of serializing it.
"""

import numpy as np
import ml_dtypes

import concourse.bacc as bacc
import concourse.tile as tile
import concourse.mybir as mybir
from concourse.bass_utils import run_bass_kernel_spmd

FP32 = mybir.dt.float32
BF16 = mybir.dt.bfloat16
AF = mybir.ActivationFunctionType
OP = mybir.AluOpType

B, DM, D, N, R, K = 2, 96, 192, 16, 6, 2
DHALF = 96
NHALF = 8
NCORES = 8


def diag_order(size):
    locs = [i * size + i for i in range(size)]
    for i in range(size):
        for j in range(i + 1, size):
            locs.append(i * size + j)
    for i in range(size):
        for j in range(i):
            locs.append(i * size + j)
    return np.asarray(locs, dtype=np.int64)


def diag_segments(H):
    segs = [(0, 0, H, H + 1)]
    p = H
    for i in range(H):
        ln = H - 1 - i
        if ln > 0:
            segs.append((p, i * H + i + 1, ln, 1))
            p += ln
    for i in range(H):
        ln = i
        if ln > 0:
            segs.append((p, i * H, ln, 1))
            p += ln
    assert p == H * H
    return segs


_PROGRAMS = {}


def build_program(H=64):
    W = H
    L = H * W
    LQ = L // 4
    FC = min(512, L)
    NF = L // FC
    RPC = FC // W
    FCP = min(512, LQ)
    segs = diag_segments(H)

    nc = bacc.Bacc("TRN2", target_bir_lowering=False, debug=False,
                   enable_asserts=False, num_devices=NCORES)

    def din(name, shape, dt=BF16):
        return nc.dram_tensor(name, shape, dt, kind="ExternalInput").ap()

    i_xT = din("xT", [96, L])
    i_xTq = din("xTq", [96, LQ])
    i_wxi = din("wxi", [96, 2 * 96])
    i_wd = din("wd", [96, 2 * 10 * 96])
    i_cb = din("cb", [96, 2], FP32)
    i_xpj = din("xpj", [96, 2 * 44])
    i_dtw = din("dtw", [12, 96])
    i_dtb = din("dtb", [96, 2], FP32)
    i_A = din("Acol", [96, 16], FP32)
    i_dse = din("dse", [96, 1], FP32)
    i_mlo = din("mlo", [96, 1], FP32)
    i_mhi = din("mhi", [96, 1], FP32)
    i_id96 = din("id96", [96, 96])
    i_onc = din("onc", [96, 1])
    i_onr = din("onr", [1, 96])
    i_lng = din("lng", [96, 2], FP32)
    i_lnb = din("lnb", [96, 2], FP32)
    i_wz = din("wz", [96, 192])
    i_wo = din("wo", [96, 192])
    o_out = nc.dram_tensor("out_part", [96, LQ], FP32, kind="ExternalOutput").ap()

    RG_QUAD = [[0, 1, 2, 3], [4, 5, 6, 7]]

    with tile.TileContext(nc) as tc:
        with tc.tile_pool(name="cst", bufs=1) as cst, \
             tc.tile_pool(name="big", bufs=1) as big, \
             tc.tile_pool(name="tmp", bufs=2) as tmp, \
             tc.tile_pool(name="kbuf", bufs=1) as kbuf, \
             tc.tile_pool(name="nlp", bufs=2) as nlp, \
             tc.tile_pool(name="pst", bufs=1) as pst, \
             tc.tile_pool(name="ps", bufs=1, space="PSUM") as ps, \
             tc.tile_pool(name="drm", bufs=1, space="DRAM") as drm:

            ps_ctr = [0]

            def ps_tile(shape):
                t = ps.tile(shape, FP32, tag=f"y{ps_ctr[0] % 8}")
                ps_ctr[0] += 1
                return t

            def load(ap_in, shape, dt=BF16, pool=cst, nm=None):
                nm = nm or f"c_{ap_in.tensor.name}"
                t = pool.tile(shape, dt, tag=nm, name=nm)
                nc.sync.dma_start(out=t, in_=ap_in)
                return t

            # ---- constants
            t_wxi = load(i_wxi, [96, 2, 96])
            t_wd = load(i_wd, [96, 2, 10, 96])
            t_cb = load(i_cb, [96, 2], FP32)
            t_xpj = load(i_xpj, [96, 2, 44])
            t_dtw0 = load(i_dtw[0:6, :], [6, 96], nm='c_dtw0')
            t_dtw1 = load(i_dtw[6:12, :], [6, 96], nm='c_dtw1')
            t_dtws = [t_dtw0, t_dtw1]
            t_dtb = load(i_dtb, [96, 2], FP32)
            t_A = load(i_A, [96, 16], FP32)
            t_dse = load(i_dse, [96, 1], FP32)
            t_mlo = load(i_mlo, [96, 1], FP32)
            t_mhi = load(i_mhi, [96, 1], FP32)
            t_id96 = load(i_id96, [96, 96])
            t_onc = load(i_onc, [96, 1])
            t_onr = load(i_onr, [1, 96])
            t_lng = load(i_lng, [96, 2], FP32)
            t_lnb = load(i_lnb, [96, 2], FP32)
            t_wz = load(i_wz, [96, 2, 96])
            t_wo = load(i_wo, [96, 2, 96])
            t_xTq = load(i_xTq, [96, LQ])
            t_eps = cst.tile([1, 1], FP32)
            nc.vector.memset(t_eps, 1e-5)
            t_ones = cst.tile([96, FC], BF16)
            nc.gpsimd.memset(t_ones, 1.0)

            # ---- phases B-E, per d-half (h=0 is this core's own half)
            t_xT = tmp.tile([96, L], BF16, tag="tmp16")
            nc.sync.dma_start(out=t_xT, in_=i_xT)
            d_xd = drm.tile([44, L], BF16)
            t_xs0s = []
            for h in (0, 1):
                t_xpad = tmp.tile([96, H + 2, W + 2], BF16, tag="tmp16",
                                  name=f"t_xpad{h}")
                nc.gpsimd.memset(t_xpad, 0.0)
                for f in range(NF):
                    p = ps_tile([96, FC])
                    nc.tensor.matmul(p, t_wxi[:, h, :],
                                     t_xT[:, f * FC:(f + 1) * FC],
                                     start=True, stop=True)
                    nc.vector.tensor_copy(
                        out=t_xpad[:, 1 + f * RPC:1 + (f + 1) * RPC, 1:W + 1],
                        in_=p.rearrange("p (r w) -> p r w", w=W))
                t_xsr = tmp.tile([96, L], BF16, tag="tmp16", name=f"t_xsr{h}")
                for f in range(NF):
                    p = ps_tile([96, RPC, W])
                    for t in range(9):
                        ky, kx = t // 3, t % 3
                        rhs = t_xpad[:, f * RPC + ky:f * RPC + ky + RPC,
                                     kx:kx + W]
                        nc.tensor.matmul(p, t_wd[:, h, t, :], rhs,
                                         start=(t == 0), stop=False)
                    nc.tensor.matmul(p, t_wd[:, h, 9, :],
                                     t_ones.rearrange("p (r w) -> p r w", w=W),
                                     start=False, stop=True)
                    p2 = p.rearrange("p r w -> p (r w)")
                    fs = slice(f * FC, (f + 1) * FC)
                    t_cv = nlp.tile([96, FC], BF16, tag="cv")
                    nc.scalar.activation(out=t_cv, in_=p2, func=AF.Identity,
                                         bias=0.0, scale=1.0)
                    t_sgc = nlp.tile([96, FC], BF16, tag="sgc")
                    nc.scalar.activation(out=t_sgc, in_=p2, func=AF.Sigmoid,
                                         bias=0.0, scale=1.0)
                    nc.vector.tensor_tensor(out=t_xsr[:, fs], in0=t_cv,
                                            in1=t_sgc, op=OP.mult)
                t_xs0 = big.tile([96, L], BF16, tag=f"xs0_{h}", name=f"t_xs0_{h}")
                for si, (dp, rp, ln, st) in enumerate(segs):
                    sg = t_xsr[:, rp:rp + (ln - 1) * st + 1:st] if st > 1 \
                        else t_xsr[:, rp:rp + ln]
                    if (si * 2 + h) % 5 < 3:
                        nc.vector.tensor_copy(out=t_xs0[:, dp:dp + ln], in_=sg)
                    else:
                        nc.gpsimd.tensor_copy(out=t_xs0[:, dp:dp + ln], in_=sg)
                t_xs0s.append(t_xs0)
            t_xs0 = t_xs0s[0]

            # ---- phase E: x_proj -> DRAM, dt expansion, sigmoid
            t_sgf = []
            for k in (0, 1):
                t_s = tmp.tile([96, L], BF16, tag="tmp16", name=f"t_sgf{k}")
                t_sgf.append(t_s)
            for f in range(NF):
                fs = slice(f * FC, (f + 1) * FC)
                for k in (0, 1):
                    p = ps_tile([22, FC])
                    for h in (0, 1):
                        nc.tensor.matmul(p, t_xpj[:, h, k * 22:(k + 1) * 22],
                                         t_xs0s[h][:, fs],
                                         start=(h == 0), stop=(h == 1))
                    t_xdc = nlp.tile([22, FC], BF16, tag="xdc", bufs=2)
                    if (f + k) % 2 == 0:
                        nc.vector.tensor_copy(out=t_xdc, in_=p)
                    else:
                        nc.scalar.copy(out=t_xdc, in_=p)
                    nc.sync.dma_start(out=d_xd[k * 22:k * 22 + 22, fs],
                                      in_=t_xdc)
                    p2 = ps_tile([96, FC])
                    nc.tensor.matmul(p2, t_dtws[k], t_xdc[0:6, :],
                                     start=True, stop=True)
                    nc.scalar.activation(out=t_sgf[k][:, fs],
                                         in_=p2, func=AF.Sigmoid,
                                         bias=t_dtb[:, k:k + 1], scale=-1.0)

            # ---- phase G: -delta = ln(sigmoid), du = -delta*xs0
            t_delta = []
            t_du = []
            for k in (0, 1):
                nc.scalar.activation(out=t_sgf[k], in_=t_sgf[k], func=AF.Ln,
                                     bias=0.0, scale=1.0)
                t_delta.append(t_sgf[k])
                t_d = kbuf.tile([96, L], BF16, tag=f"du{k}", name=f"t_du{k}")
                nc.vector.tensor_tensor(out=t_d, in0=t_delta[k], in1=t_xs0,
                                        op=OP.mult)
                t_du.append(t_d)

            # ---- scan loop: k=1 first, evict + RS#1 (hidden under k=0),
            # then k=0, evict + RS#2 (exposed).
            d_s1 = drm.tile([4, D, LQ], BF16)
            d_r1 = drm.tile([D, LQ], BF16)
            d_s2 = drm.tile([4, D, LQ], BF16)
            d_r2 = drm.tile([D, LQ], BF16)
            d_rs = [d_s2, d_s1]

            while ps_ctr[0] % 8 != 0:
                ps_ctr[0] += 1

            for k in (1, 0):
                yps = [ps.tile([96, FC], FP32, tag=f"y{f}", name=f"yps{k}{f}")
                       for f in range(NF)]
                for j in range(NHALF):
                    row_b = k * 22 + 6 + j
                    row_c = k * 22 + 14 + j
                    t_br = nlp.tile([96, L], BF16, tag="brep", bufs=2)
                    t_cr = nlp.tile([96, L], BF16, tag="crep", bufs=2)
                    nc.sync.dma_start(
                        out=t_br,
                        in_=d_xd[row_b:row_b + 1, :].to_broadcast((96, L)))
                    nc.sync.dma_start(
                        out=t_cr,
                        in_=d_xd[row_c:row_c + 1, :].to_broadcast((96, L)))
                    t_dA = nlp.tile([96, L], BF16, tag="dA", bufs=2)
                    nc.scalar.activation(out=t_dA, in_=t_delta[k], func=AF.Exp,
                                         bias=0.0,
                                         scale=t_A[:, k * 8 + j:k * 8 + j + 1])
                    t_dBu = nlp.tile([96, L], BF16, tag="dBu", bufs=2)
                    nc.gpsimd.tensor_tensor(out=t_dBu, in0=t_du[k], in1=t_br,
                                            op=OP.mult)
                    t_h = nlp.tile([96, L], BF16, tag="h", bufs=2)
                    if k == 0:
                        nc.vector.tensor_tensor_scan(
                            out=t_h, data0=t_dA, data1=t_dBu, initial=0.0,
                            op0=OP.mult, op1=OP.add)
                    else:
                        nc.vector.tensor_tensor_scan(
                            out=t_h[:, ::-1], data0=t_dA[:, ::-1],
                            data1=t_dBu[:, ::-1], initial=0.0,
                            op0=OP.mult, op1=OP.add)
                    LS = L // 2
                    nc.vector.tensor_tensor(out=t_h[:, :LS], in0=t_h[:, :LS],
                                            in1=t_cr[:, :LS], op=OP.mult)
                    nc.gpsimd.tensor_tensor(out=t_h[:, LS:], in0=t_h[:, LS:],
                                            in1=t_cr[:, LS:], op=OP.mult)
                    for f in range(NF):
                        nc.tensor.matmul(yps[f], t_id96,
                                         t_h[:, f * FC:(f + 1) * FC],
                                         start=(j == 0), stop=(j == NHALF - 1))
                # evict this direction's partial y (masked into global-d rows)
                for f in range(NF):
                    q, half = f // 2, (f % 2) * FC
                    cs = slice(half, half + FC)
                    t_lo = nlp.tile([96, FC], BF16, tag="ylo", bufs=2)
                    t_hi = nlp.tile([96, FC], BF16, tag="yhi2", bufs=2)
                    if k == 1:
                        nc.scalar.activation(out=t_lo, in_=yps[f],
                                             func=AF.Identity, bias=0.0,
                                             scale=t_mlo)
                        nc.scalar.activation(out=t_hi, in_=yps[f],
                                             func=AF.Identity, bias=0.0,
                                             scale=t_mhi)
                    else:
                        t_yev = nlp.tile([96, FC], BF16, tag="yev", bufs=1)
                        nc.vector.scalar_tensor_tensor(
                            out=t_yev, in0=t_xs0[:, f * FC:(f + 1) * FC],
                            scalar=t_dse, in1=yps[f], op0=OP.mult, op1=OP.add)
                        nc.vector.tensor_scalar_mul(out=t_lo, in0=t_yev,
                                                    scalar1=t_mlo)
                        nc.scalar.activation(out=t_hi, in_=t_yev,
                                             func=AF.Identity, bias=0.0,
                                             scale=t_mhi)
                    nc.sync.dma_start(out=d_rs[k][q, 0:96, cs], in_=t_lo)
                    nc.sync.dma_start(out=d_rs[k][q, 96:192, cs], in_=t_hi)
                if k == 1:
                    nc.gpsimd.collective_compute(
                        "ReduceScatter", OP.add, replica_groups=RG_QUAD,
                        ins=[d_s1.opt()], outs=[d_r1.opt()])
            nc.gpsimd.collective_compute(
                "ReduceScatter", OP.add, replica_groups=RG_QUAD,
                ins=[d_s2.opt()], outs=[d_r2.opt()])

            # ---- z-path (independent; fills the RS#2 wait window)
            t_zs = []
            for h in (0, 1):
                t_z = pst.tile([96, LQ], BF16, tag=f"z{h}", name=f"t_z{h}")
                for f in range(LQ // FCP):
                    p = ps_tile([96, FCP])
                    nc.tensor.matmul(p, t_wz[:, h, :],
                                     t_xTq[:, f * FCP:(f + 1) * FCP],
                                     start=True, stop=True)
                    fs = slice(f * FCP, (f + 1) * FCP)
                    t_z1 = nlp.tile([96, FCP], BF16, tag="z1", bufs=1)
                    nc.scalar.activation(out=t_z1, in_=p, func=AF.Identity,
                                         bias=0.0, scale=1.0)
                    t_zg = nlp.tile([96, FCP], BF16, tag="zg", bufs=1)
                    nc.scalar.activation(out=t_zg, in_=p, func=AF.Sigmoid,
                                         bias=0.0, scale=1.0)
                    nc.vector.tensor_tensor(out=t_z[:, fs], in0=t_z1,
                                            in1=t_zg, op=OP.mult)
                t_zs.append(t_z)

            # ---- combine the two RS results
            t_yh = []
            for h in (0, 1):
                t_a = nlp.tile([96, LQ], BF16, tag="rca", bufs=1)
                nc.sync.dma_start(out=t_a, in_=d_r1[h * 96:(h + 1) * 96, :])
                t_b = nlp.tile([96, LQ], BF16, tag="rcb", bufs=1)
                nc.sync.dma_start(out=t_b, in_=d_r2[h * 96:(h + 1) * 96, :])
                t_y = pst.tile([96, LQ], BF16, tag=f"yh{h}", name=f"t_yh{h}")
                eng = nc.vector if h == 0 else nc.gpsimd
                eng.tensor_tensor(out=t_y, in0=t_a, in1=t_b, op=OP.add)
                t_yh.append(t_y)

            # ---- phase I: post (LayerNorm over 192 ch, gate, out-proj),
            # all in diag order; host un-permutes.
            t_out = pst.tile([96, LQ], FP32, tag="outp")
            for f in range(LQ // FCP):
                fs = slice(f * FCP, (f + 1) * FCP)
                p_s = ps_tile([1, FCP])
                p_m = ps_tile([1, FCP])
                for h in (0, 1):
                    t_y2 = nlp.tile([96, FCP], BF16, tag="y2", bufs=1)
                    nc.scalar.activation(out=t_y2, in_=t_yh[h][:, fs],
                                         func=AF.Square)
                    nc.tensor.matmul(p_s, t_onc, t_yh[h][:, fs],
                                     start=(h == 0), stop=(h == 1))
                    nc.tensor.matmul(p_m, t_onc, t_y2,
                                     start=(h == 0), stop=(h == 1))
                t_mu = nlp.tile([1, FCP], FP32, tag="mu", bufs=1)
                nc.vector.tensor_scalar_mul(out=t_mu, in0=p_s, scalar1=1.0 / D)
                t_m2 = nlp.tile([1, FCP], FP32, tag="m2", bufs=1)
                nc.vector.tensor_scalar_mul(out=t_m2, in0=p_m, scalar1=1.0 / D)
                t_mu2 = nlp.tile([1, FCP], FP32, tag="mu2", bufs=1)
                nc.vector.tensor_tensor(out=t_mu2, in0=t_mu, in1=t_mu,
                                        op=OP.mult)
                t_var = nlp.tile([1, FCP], FP32, tag="var", bufs=1)
                nc.vector.tensor_tensor(out=t_var, in0=t_m2, in1=t_mu2,
                                        op=OP.subtract)
                t_std = nlp.tile([1, FCP], FP32, tag="std", bufs=1)
                nc.scalar.activation(out=t_std, in_=t_var, func=AF.Sqrt,
                                     bias=t_eps)
                t_rstd = nlp.tile([1, FCP], FP32, tag="rstd", bufs=1)
                nc.vector.reciprocal(out=t_rstd, in_=t_std)
                t_mub = nlp.tile([1, FCP], BF16, tag="mub", bufs=1)
                nc.vector.tensor_copy(out=t_mub, in_=t_mu)
                t_rsb = nlp.tile([1, FCP], BF16, tag="rsb", bufs=1)
                nc.vector.tensor_copy(out=t_rsb, in_=t_rstd)
                p_bm = ps_tile([96, FCP])
                nc.tensor.matmul(p_bm, t_onr, t_mub, start=True, stop=True)
                p_br = ps_tile([96, FCP])
                nc.tensor.matmul(p_br, t_onr, t_rsb, start=True, stop=True)
                p_o = ps_tile([96, FCP])
                for h in (0, 1):
                    t_c1 = nlp.tile([96, FCP], BF16, tag="c1", bufs=1)
                    nc.vector.tensor_tensor(out=t_c1, in0=t_yh[h][:, fs],
                                            in1=p_bm, op=OP.subtract)
                    t_c2 = nlp.tile([96, FCP], BF16, tag="c2", bufs=1)
                    nc.vector.tensor_tensor(out=t_c2, in0=t_c1, in1=p_br,
                                            op=OP.mult)
                    t_c3 = nlp.tile([96, FCP], BF16, tag="c3", bufs=1)
                    nc.vector.tensor_scalar(out=t_c3, in0=t_c2,
                                            scalar1=t_lng[:, h:h + 1],
                                            scalar2=t_lnb[:, h:h + 1],
                                            op0=OP.mult, op1=OP.add)
                    t_ygt = nlp.tile([96, FCP], BF16, tag=f"yg{h}")
                    nc.vector.tensor_tensor(out=t_ygt, in0=t_c3,
                                            in1=t_zs[h][:, fs], op=OP.mult)
                    nc.tensor.matmul(p_o, t_wo[:, h, :], t_ygt,
                                     start=(h == 0), stop=(h == 1))
                nc.vector.tensor_copy(out=t_out[:, fs], in_=p_o)
            nc.sync.dma_start(out=o_out, in_=t_out)

    nc.finalize()
    return nc


def get_program(H=64):
    if H not in _PROGRAMS:
        _PROGRAMS[H] = build_program(H)
    return _PROGRAMS[H]


# ---------------------------------------------------------------- host side

def make_in_maps(inputs, H=64):
    W = H
    L = H * W
    LQ = L // 4
    bf = ml_dtypes.bfloat16
    order = diag_order(H)

    x = np.asarray(inputs["x"], np.float32)
    w_in = np.asarray(inputs["w_in"], np.float32)
    conv_w = np.asarray(inputs["conv_w"], np.float32)
    conv_b = np.asarray(inputs["conv_b"], np.float32)
    x_proj_w = np.asarray(inputs["x_proj_w"], np.float32)
    dt_w = np.asarray(inputs["dt_w"], np.float32)
    dt_b = np.asarray(inputs["dt_b"], np.float32)
    A_logs = np.asarray(inputs["A_logs"], np.float32)
    Ds = np.asarray(inputs["Ds"], np.float32)
    ln_g = np.asarray(inputs["ln_g"], np.float32)
    ln_b = np.asarray(inputs["ln_b"], np.float32)
    w_out = np.asarray(inputs["w_out"], np.float32)

    A_full = np.exp(A_logs).reshape(K, D, N)
    Ds2 = Ds.reshape(K, D)
    id96 = np.eye(96, dtype=np.float32)
    onc = np.ones((96, 1), np.float32)
    onr = np.ones((1, 96), np.float32)

    in_maps = []
    for c in range(NCORES):
        b, dh, nh = c // 4, (c // 2) % 2, c % 2
        lq = dh * 2 + nh
        dsl = slice(dh * 96, dh * 96 + 96)
        nsl = slice(nh * 8, nh * 8 + 8)

        xT = np.ascontiguousarray(x[b].reshape(L, DM).T)
        xTq = np.ascontiguousarray(xT[:, order[lq * LQ:(lq + 1) * LQ]])

        halves = [dsl, slice((1 - dh) * 96, (1 - dh) * 96 + 96)]
        wd = np.zeros((96, 2, 10, 96), np.float32)
        xpj = np.zeros((96, 2, 44), np.float32)
        wxi2 = np.zeros((96, 2, 96), np.float32)
        cb2 = np.zeros((96, 2), np.float32)
        for hh, hsl in enumerate(halves):
            cw = conv_w[hsl, 0].reshape(96, 9)
            for t in range(9):
                np.fill_diagonal(wd[:, hh, t, :], cw[:, t])
            np.fill_diagonal(wd[:, hh, 9, :], conv_b[hsl])
            wxi2[:, hh, :] = w_in[hsl, :].T
            cb2[:, hh] = conv_b[hsl]
            for k in range(K):
                sel = np.concatenate([
                    x_proj_w[k, 0:R, hsl],
                    -x_proj_w[k, R + nh * 8:R + nh * 8 + 8, hsl],
                    x_proj_w[k, R + N + nh * 8:R + N + nh * 8 + 8, hsl],
                ], axis=0)
                xpj[:, hh, k * 22:(k + 1) * 22] = sel.T

        dtw = np.zeros((12, 96), np.float32)
        for k in range(K):
            dtw[k * 6:(k + 1) * 6, :] = dt_w[k, dsl, :].T

        dtb = -np.stack([dt_b[0, dsl], dt_b[1, dsl]], axis=1)
        Acol = np.concatenate([A_full[0, dsl, nsl], A_full[1, dsl, nsl]], axis=1)
        dse = ((Ds2[0, dsl] + Ds2[1, dsl]) / 2.0)[:, None]
        lng = np.stack([ln_g[0:96], ln_g[96:192]], axis=1)
        lnb = np.stack([ln_b[0:96], ln_b[96:192]], axis=1)
        wz = np.ascontiguousarray(w_in[D:2 * D, :].T)
        wo = np.concatenate([w_out[:, 0:96].T, w_out[:, 96:192].T], axis=1)

        in_maps.append({
            "xT": xT.astype(bf),
            "xTq": xTq.astype(bf),
            "wxi": wxi2.reshape(96, 2 * 96).astype(bf),
            "wd": wd.reshape(96, 2 * 10 * 96).astype(bf),
            "cb": cb2.astype(np.float32),
            "xpj": xpj.reshape(96, 2 * 44).astype(bf),
            "dtw": dtw.astype(bf),
            "dtb": dtb.astype(np.float32),
            "Acol": Acol.astype(np.float32),
            "dse": dse.astype(np.float32),
            "mlo": np.full((96, 1), 1.0 if dh == 0 else 0.0, np.float32),
            "mhi": np.full((96, 1), 1.0 if dh == 1 else 0.0, np.float32),
            "id96": id96.astype(bf),
            "onc": onc.astype(bf),
            "onr": onr.astype(bf),
            "lng": lng.astype(np.float32),
            "lnb": lnb.astype(np.float32),
            "wz": wz.astype(bf),
            "wo": wo.astype(bf),
        })
    return in_maps


def assemble_output(results, H=64):
    L = H * H
    LQ = L // 4
    order = diag_order(H)
    out = np.zeros((B, L, DM), np.float32)
    for c in range(NCORES):
        b, dh, nh = c // 4, (c // 2) % 2, c % 2
        lq = dh * 2 + nh
        out[b, order[lq * LQ:(lq + 1) * LQ], :] = results[c]["out_part"].T
    return out.reshape(B, H, H, DM)


def kernel(**inputs):
    nc = get_program(64)
    in_maps = make_in_maps(inputs, 64)
    res = run_bass_kernel_spmd(nc, in_maps, core_ids=list(range(NCORES)))
    return assemble_output(res.results, 64)


# revision 24
# speedup vs baseline: 1.1995x; 1.1995x over previous
"""SS2D (VMamba-style 2D selective scan) Trainium2 Bass kernel.

Full inputs in, full output out. Internally: 8-core SPMD.

Sharding: core c = (b, dh, nh):  b = c//4, dh = (c//2)%2, nh = c%2.
  - b: batch.
  - dh: d_inner half (96 channels of 192).
  - nh: d_state half (8 of 16 states), processed for BOTH scan directions k.

v3 structure:
  - Everything after the conv stays in DIAGONAL order; the host un-permutes
    the output (and pre-gathers the z-gate input), so the on-device un-diag
    reorder pass is gone.
  - The scan loop runs direction k=1 first, evicts its partial y, and ships
    it through a ReduceScatter that overlaps the whole k=0 compute phase;
    only the k=0 ReduceScatter is exposed.
  - B/C broadcast DMAs are issued from two different queues (sync/scalar)
    with deeper buffering and no in-place aliasing, so they pipeline with
    the scan instead of serializing it.
"""

import numpy as np
import ml_dtypes

import concourse.bacc as bacc
import concourse.tile as tile
import concourse.mybir as mybir
from concourse.bass_utils import run_bass_kernel_spmd

FP32 = mybir.dt.float32
BF16 = mybir.dt.bfloat16
AF = mybir.ActivationFunctionType
OP = mybir.AluOpType

B, DM, D, N, R, K = 2, 96, 192, 16, 6, 2
DHALF = 96
NHALF = 8
NCORES = 8


def diag_order(size):
    locs = [i * size + i for i in range(size)]
    for i in range(size):
        for j in range(i + 1, size):
            locs.append(i * size + j)
    for i in range(size):
        for j in range(i):
            locs.append(i * size + j)
    return np.asarray(locs, dtype=np.int64)


def diag_segments(H):
    segs = [(0, 0, H, H + 1)]
    p = H
    for i in range(H):
        ln = H - 1 - i
        if ln > 0:
            segs.append((p, i * H + i + 1, ln, 1))
            p += ln
    for i in range(H):
        ln = i
        if ln > 0:
            segs.append((p, i * H, ln, 1))
            p += ln
    assert p == H * H
    return segs


_PROGRAMS = {}


def build_program(H=64):
    W = H
    L = H * W
    LQ = L // 4
    FC = min(512, L)
    NF = L // FC
    RPC = FC // W
    FCP = min(512, LQ)
    segs = diag_segments(H)

    nc = bacc.Bacc("TRN2", target_bir_lowering=False, debug=False,
                   enable_asserts=False, num_devices=NCORES)

    def din(name, shape, dt=BF16):
        return nc.dram_tensor(name, shape, dt, kind="ExternalInput").ap()

    i_xT = din("xT", [96, L])
    i_xTq = din("xTq", [96, LQ])
    i_wxi = din("wxi", [96, 2 * 96])
    i_wd = din("wd", [96, 2 * 10 * 96])
    i_cb = din("cb", [96, 2], FP32)
    i_xpj = din("xpj", [96, 2 * 44])
    i_dtw = din("dtw", [12, 96])
    i_dtb = din("dtb", [96, 2], FP32)
    i_A = din("Acol", [96, 16], FP32)
    i_dse = din("dse", [96, 1], FP32)
    i_mlo = din("mlo", [96, 1], FP32)
    i_mhi = din("mhi", [96, 1], FP32)
    i_id96 = din("id96", [96, 96])
    i_onc = din("onc", [96, 1])
    i_onr = din("onr", [1, 96])
    i_lng = din("lng", [96, 2], FP32)
    i_lnb = din("lnb", [96, 2], FP32)
    i_wz = din("wz", [96, 192])
    i_wo = din("wo", [96, 192])
    o_out = nc.dram_tensor("out_part", [96, LQ], FP32, kind="ExternalOutput").ap()

    RG_QUAD = [[0, 1, 2, 3], [4, 5, 6, 7]]

    with tile.TileContext(nc) as tc:
        with tc.tile_pool(name="cst", bufs=1) as cst, \
             tc.tile_pool(name="big", bufs=1) as big, \
             tc.tile_pool(name="tmp", bufs=2) as tmp, \
             tc.tile_pool(name="kbuf", bufs=1) as kbuf, \
             tc.tile_pool(name="nlp", bufs=2) as nlp, \
             tc.tile_pool(name="pst", bufs=1) as pst, \
             tc.tile_pool(name="ps", bufs=1, space="PSUM") as ps, \
             tc.tile_pool(name="drm", bufs=1, space="DRAM") as drm:

            ps_ctr = [0]

            def ps_tile(shape):
                t = ps.tile(shape, FP32, tag=f"y{ps_ctr[0] % 8}")
                ps_ctr[0] += 1
                return t

            def load(ap_in, shape, dt=BF16, pool=cst, nm=None):
                nm = nm or f"c_{ap_in.tensor.name}"
                t = pool.tile(shape, dt, tag=nm, name=nm)
                nc.sync.dma_start(out=t, in_=ap_in)
                return t

            # ---- constants
            t_wxi = load(i_wxi, [96, 2, 96])
            t_wd = load(i_wd, [96, 2, 10, 96])
            t_cb = load(i_cb, [96, 2], FP32)
            t_xpj = load(i_xpj, [96, 2, 44])
            t_dtw0 = load(i_dtw[0:6, :], [6, 96], nm='c_dtw0')
            t_dtw1 = load(i_dtw[6:12, :], [6, 96], nm='c_dtw1')
            t_dtws = [t_dtw0, t_dtw1]
            t_dtb = load(i_dtb, [96, 2], FP32)
            t_A = load(i_A, [96, 16], FP32)
            t_dse = load(i_dse, [96, 1], FP32)
            t_mlo = load(i_mlo, [96, 1], FP32)
            t_mhi = load(i_mhi, [96, 1], FP32)
            t_id96 = load(i_id96, [96, 96])
            t_onc = load(i_onc, [96, 1])
            t_onr = load(i_onr, [1, 96])
            t_lng = load(i_lng, [96, 2], FP32)
            t_lnb = load(i_lnb, [96, 2], FP32)
            t_wz = load(i_wz, [96, 2, 96])
            t_wo = load(i_wo, [96, 2, 96])
            t_xTq = load(i_xTq, [96, LQ])
            t_eps = cst.tile([1, 1], FP32)
            nc.vector.memset(t_eps, 1e-5)
            t_ones = cst.tile([96, FC], BF16)
            nc.gpsimd.memset(t_ones, 1.0)

            # ---- phases B-E, per d-half (h=0 is this core's own half)
            t_xT = tmp.tile([96, L], BF16, tag="tmp16")
            nc.sync.dma_start(out=t_xT, in_=i_xT)
            d_xd = drm.tile([44, L], BF16)
            t_xs0s = []
            for h in (0, 1):
                t_xpad = tmp.tile([96, H + 2, W + 2], BF16, tag="tmp16",
                                  name=f"t_xpad{h}")
                nc.gpsimd.memset(t_xpad, 0.0)
                for f in range(NF):
                    p = ps_tile([96, FC])
                    nc.tensor.matmul(p, t_wxi[:, h, :],
                                     t_xT[:, f * FC:(f + 1) * FC],
                                     start=True, stop=True)
                    nc.vector.tensor_copy(
                        out=t_xpad[:, 1 + f * RPC:1 + (f + 1) * RPC, 1:W + 1],
                        in_=p.rearrange("p (r w) -> p r w", w=W))
                t_xsr = tmp.tile([96, L], BF16, tag="tmp16", name=f"t_xsr{h}")
                for f in range(NF):
                    p = ps_tile([96, RPC, W])
                    for t in range(9):
                        ky, kx = t // 3, t % 3
                        rhs = t_xpad[:, f * RPC + ky:f * RPC + ky + RPC,
                                     kx:kx + W]
                        nc.tensor.matmul(p, t_wd[:, h, t, :], rhs,
                                         start=(t == 0), stop=False)
                    nc.tensor.matmul(p, t_wd[:, h, 9, :],
                                     t_ones.rearrange("p (r w) -> p r w", w=W),
                                     start=False, stop=True)
                    p2 = p.rearrange("p r w -> p (r w)")
                    fs = slice(f * FC, (f + 1) * FC)
                    t_cv = nlp.tile([96, FC], BF16, tag="cv")
                    nc.scalar.activation(out=t_cv, in_=p2, func=AF.Identity,
                                         bias=0.0, scale=1.0)
                    t_sgc = nlp.tile([96, FC], BF16, tag="sgc")
                    nc.scalar.activation(out=t_sgc, in_=p2, func=AF.Sigmoid,
                                         bias=0.0, scale=1.0)
                    nc.vector.tensor_tensor(out=t_xsr[:, fs], in0=t_cv,
                                            in1=t_sgc, op=OP.mult)
                t_xs0 = big.tile([96, L], BF16, tag=f"xs0_{h}", name=f"t_xs0_{h}")
                for si, (dp, rp, ln, st) in enumerate(segs):
                    sg = t_xsr[:, rp:rp + (ln - 1) * st + 1:st] if st > 1 \
                        else t_xsr[:, rp:rp + ln]
                    if (si * 2 + h) % 5 < 3:
                        nc.vector.tensor_copy(out=t_xs0[:, dp:dp + ln], in_=sg)
                    else:
                        nc.gpsimd.tensor_copy(out=t_xs0[:, dp:dp + ln], in_=sg)
                t_xs0s.append(t_xs0)
            t_xs0 = t_xs0s[0]

            # ---- phase E: x_proj -> DRAM, dt expansion, sigmoid
            t_sgf = []
            for k in (0, 1):
                t_s = tmp.tile([96, L], BF16, tag="tmp16", name=f"t_sgf{k}")
                t_sgf.append(t_s)
            for f in range(NF):
                fs = slice(f * FC, (f + 1) * FC)
                for k in (0, 1):
                    p = ps_tile([22, FC])
                    for h in (0, 1):
                        nc.tensor.matmul(p, t_xpj[:, h, k * 22:(k + 1) * 22],
                                         t_xs0s[h][:, fs],
                                         start=(h == 0), stop=(h == 1))
                    t_xdc = nlp.tile([22, FC], BF16, tag="xdc", bufs=2)
                    if (f + k) % 2 == 0:
                        nc.vector.tensor_copy(out=t_xdc, in_=p)
                    else:
                        nc.scalar.copy(out=t_xdc, in_=p)
                    nc.sync.dma_start(out=d_xd[k * 22:k * 22 + 22, fs],
                                      in_=t_xdc)
                    p2 = ps_tile([96, FC])
                    nc.tensor.matmul(p2, t_dtws[k], t_xdc[0:6, :],
                                     start=True, stop=True)
                    nc.scalar.activation(out=t_sgf[k][:, fs],
                                         in_=p2, func=AF.Sigmoid,
                                         bias=t_dtb[:, k:k + 1], scale=-1.0)

            # ---- phase G: -delta = ln(sigmoid), du = -delta*xs0
            t_delta = []
            t_du = []
            for k in (0, 1):
                nc.scalar.activation(out=t_sgf[k], in_=t_sgf[k], func=AF.Ln,
                                     bias=0.0, scale=1.0)
                t_delta.append(t_sgf[k])
                t_d = kbuf.tile([96, L], BF16, tag=f"du{k}", name=f"t_du{k}")
                nc.vector.tensor_tensor(out=t_d, in0=t_delta[k], in1=t_xs0,
                                        op=OP.mult)
                t_du.append(t_d)

            # ---- scan loop: k=1 first, evict + RS#1 (hidden under k=0),
            # then k=0, evict + RS#2 (exposed).
            d_s1 = drm.tile([4, D, LQ], BF16)
            d_r1 = drm.tile([D, LQ], BF16)
            d_s2 = drm.tile([4, D, LQ], BF16)
            d_r2 = drm.tile([D, LQ], BF16)
            d_rs = [d_s2, d_s1]

            while ps_ctr[0] % 8 != 0:
                ps_ctr[0] += 1

            for k in (1, 0):
                yps = [ps.tile([96, FC], FP32, tag=f"y{f}", name=f"yps{k}{f}")
                       for f in range(NF)]
                for j in range(NHALF):
                    row_b = k * 22 + 6 + j
                    row_c = k * 22 + 14 + j
                    t_br = nlp.tile([96, L], BF16, tag="brep", bufs=2)
                    t_cr = nlp.tile([96, L], BF16, tag="crep", bufs=2)
                    nc.sync.dma_start(
                        out=t_br,
                        in_=d_xd[row_b:row_b + 1, :].to_broadcast((96, L)))
                    ceng = nc.gpsimd if j % 2 == 0 else nc.scalar
                    ceng.dma_start(
                        out=t_cr,
                        in_=d_xd[row_c:row_c + 1, :].to_broadcast((96, L)))
                    t_dA = nlp.tile([96, L], BF16, tag="dA", bufs=2)
                    nc.scalar.activation(out=t_dA, in_=t_delta[k], func=AF.Exp,
                                         bias=0.0,
                                         scale=t_A[:, k * 8 + j:k * 8 + j + 1])
                    t_dBu = nlp.tile([96, L], BF16, tag="dBu", bufs=2)
                    nc.gpsimd.tensor_tensor(out=t_dBu, in0=t_du[k], in1=t_br,
                                            op=OP.mult)
                    t_h = nlp.tile([96, L], BF16, tag="h", bufs=2)
                    if k == 0:
                        nc.vector.tensor_tensor_scan(
                            out=t_h, data0=t_dA, data1=t_dBu, initial=0.0,
                            op0=OP.mult, op1=OP.add)
                    else:
                        nc.vector.tensor_tensor_scan(
                            out=t_h[:, ::-1], data0=t_dA[:, ::-1],
                            data1=t_dBu[:, ::-1], initial=0.0,
                            op0=OP.mult, op1=OP.add)
                    LS = (L * 3) // 4
                    nc.vector.tensor_tensor(out=t_h[:, :LS], in0=t_h[:, :LS],
                                            in1=t_cr[:, :LS], op=OP.mult)
                    nc.gpsimd.tensor_tensor(out=t_h[:, LS:], in0=t_h[:, LS:],
                                            in1=t_cr[:, LS:], op=OP.mult)
                    for f in range(NF):
                        nc.tensor.matmul(yps[f], t_id96,
                                         t_h[:, f * FC:(f + 1) * FC],
                                         start=(j == 0), stop=(j == NHALF - 1))
                # evict this direction's partial y (masked into global-d rows)
                for f in range(NF):
                    q, half = f // 2, (f % 2) * FC
                    cs = slice(half, half + FC)
                    t_lo = nlp.tile([96, FC], BF16, tag="ylo", bufs=2)
                    t_hi = nlp.tile([96, FC], BF16, tag="yhi2", bufs=2)
                    if k == 1:
                        nc.scalar.activation(out=t_lo, in_=yps[f],
                                             func=AF.Identity, bias=0.0,
                                             scale=t_mlo)
                        nc.scalar.activation(out=t_hi, in_=yps[f],
                                             func=AF.Identity, bias=0.0,
                                             scale=t_mhi)
                    else:
                        t_yev = nlp.tile([96, FC], BF16, tag="yev", bufs=1)
                        nc.vector.scalar_tensor_tensor(
                            out=t_yev, in0=t_xs0[:, f * FC:(f + 1) * FC],
                            scalar=t_dse, in1=yps[f], op0=OP.mult, op1=OP.add)
                        nc.vector.tensor_scalar_mul(out=t_lo, in0=t_yev,
                                                    scalar1=t_mlo)
                        nc.scalar.activation(out=t_hi, in_=t_yev,
                                             func=AF.Identity, bias=0.0,
                                             scale=t_mhi)
                    nc.sync.dma_start(out=d_rs[k][q, 0:96, cs], in_=t_lo)
                    nc.sync.dma_start(out=d_rs[k][q, 96:192, cs], in_=t_hi)
                if k == 1:
                    nc.gpsimd.collective_compute(
                        "ReduceScatter", OP.add, replica_groups=RG_QUAD,
                        ins=[d_s1.opt()], outs=[d_r1.opt()])
            nc.gpsimd.collective_compute(
                "ReduceScatter", OP.add, replica_groups=RG_QUAD,
                ins=[d_s2.opt()], outs=[d_r2.opt()])

            # ---- z-path (independent; fills the RS#2 wait window)
            t_zs = []
            for h in (0, 1):
                t_z = pst.tile([96, LQ], BF16, tag=f"z{h}", name=f"t_z{h}")
                for f in range(LQ // FCP):
                    p = ps_tile([96, FCP])
                    nc.tensor.matmul(p, t_wz[:, h, :],
                                     t_xTq[:, f * FCP:(f + 1) * FCP],
                                     start=True, stop=True)
                    fs = slice(f * FCP, (f + 1) * FCP)
                    t_z1 = nlp.tile([96, FCP], BF16, tag="z1", bufs=1)
                    nc.scalar.activation(out=t_z1, in_=p, func=AF.Identity,
                                         bias=0.0, scale=1.0)
                    t_zg = nlp.tile([96, FCP], BF16, tag="zg", bufs=1)
                    nc.scalar.activation(out=t_zg, in_=p, func=AF.Sigmoid,
                                         bias=0.0, scale=1.0)
                    nc.vector.tensor_tensor(out=t_z[:, fs], in0=t_z1,
                                            in1=t_zg, op=OP.mult)
                t_zs.append(t_z)

            # ---- combine the two RS results
            t_yh = []
            for h in (0, 1):
                t_a = nlp.tile([96, LQ], BF16, tag="rca", bufs=1)
                nc.sync.dma_start(out=t_a, in_=d_r1[h * 96:(h + 1) * 96, :])
                t_b = nlp.tile([96, LQ], BF16, tag="rcb", bufs=1)
                nc.sync.dma_start(out=t_b, in_=d_r2[h * 96:(h + 1) * 96, :])
                t_y = pst.tile([96, LQ], BF16, tag=f"yh{h}", name=f"t_yh{h}")
                eng = nc.vector if h == 0 else nc.gpsimd
                eng.tensor_tensor(out=t_y, in0=t_a, in1=t_b, op=OP.add)
                t_yh.append(t_y)

            # ---- phase I: post (LayerNorm over 192 ch, gate, out-proj),
            # all in diag order; host un-permutes.
            t_out = pst.tile([96, LQ], FP32, tag="outp")
            for f in range(LQ // FCP):
                fs = slice(f * FCP, (f + 1) * FCP)
                p_s = ps_tile([1, FCP])
                p_m = ps_tile([1, FCP])
                for h in (0, 1):
                    t_y2 = nlp.tile([96, FCP], BF16, tag="y2", bufs=1)
                    nc.scalar.activation(out=t_y2, in_=t_yh[h][:, fs],
                                         func=AF.Square)
                    nc.tensor.matmul(p_s, t_onc, t_yh[h][:, fs],
                                     start=(h == 0), stop=(h == 1))
                    nc.tensor.matmul(p_m, t_onc, t_y2,
                                     start=(h == 0), stop=(h == 1))
                t_mu = nlp.tile([1, FCP], FP32, tag="mu", bufs=1)
                nc.vector.tensor_scalar_mul(out=t_mu, in0=p_s, scalar1=1.0 / D)
                t_m2 = nlp.tile([1, FCP], FP32, tag="m2", bufs=1)
                nc.vector.tensor_scalar_mul(out=t_m2, in0=p_m, scalar1=1.0 / D)
                t_mu2 = nlp.tile([1, FCP], FP32, tag="mu2", bufs=1)
                nc.vector.tensor_tensor(out=t_mu2, in0=t_mu, in1=t_mu,
                                        op=OP.mult)
                t_var = nlp.tile([1, FCP], FP32, tag="var", bufs=1)
                nc.vector.tensor_tensor(out=t_var, in0=t_m2, in1=t_mu2,
                                        op=OP.subtract)
                t_std = nlp.tile([1, FCP], FP32, tag="std", bufs=1)
                nc.scalar.activation(out=t_std, in_=t_var, func=AF.Sqrt,
                                     bias=t_eps)
                t_rstd = nlp.tile([1, FCP], FP32, tag="rstd", bufs=1)
                nc.vector.reciprocal(out=t_rstd, in_=t_std)
                t_mub = nlp.tile([1, FCP], BF16, tag="mub", bufs=1)
                nc.vector.tensor_copy(out=t_mub, in_=t_mu)
                t_rsb = nlp.tile([1, FCP], BF16, tag="rsb", bufs=1)
                nc.vector.tensor_copy(out=t_rsb, in_=t_rstd)
                p_bm = ps_tile([96, FCP])
                nc.tensor.matmul(p_bm, t_onr, t_mub, start=True, stop=True)
                p_br = ps_tile([96, FCP])
                nc.tensor.matmul(p_br, t_onr, t_rsb, start=True, stop=True)
                p_o = ps_tile([96, FCP])
                for h in (0, 1):
                    t_c1 = nlp.tile([96, FCP], BF16, tag="c1", bufs=1)
                    nc.vector.tensor_tensor(out=t_c1, in0=t_yh[h][:, fs],
                                            in1=p_bm, op=OP.subtract)
                    t_c2 = nlp.tile([96, FCP], BF16, tag="c2", bufs=1)
                    nc.vector.tensor_tensor(out=t_c2, in0=t_c1, in1=p_br,
                                            op=OP.mult)
                    t_c3 = nlp.tile([96, FCP], BF16, tag="c3", bufs=1)
                    nc.vector.tensor_scalar(out=t_c3, in0=t_c2,
                                            scalar1=t_lng[:, h:h + 1],
                                            scalar2=t_lnb[:, h:h + 1],
                                            op0=OP.mult, op1=OP.add)
                    t_ygt = nlp.tile([96, FCP], BF16, tag=f"yg{h}")
                    nc.vector.tensor_tensor(out=t_ygt, in0=t_c3,
                                            in1=t_zs[h][:, fs], op=OP.mult)
                    nc.tensor.matmul(p_o, t_wo[:, h, :], t_ygt,
                                     start=(h == 0), stop=(h == 1))
                nc.vector.tensor_copy(out=t_out[:, fs], in_=p_o)
            nc.sync.dma_start(out=o_out, in_=t_out)

    nc.finalize()
    return nc


def get_program(H=64):
    if H not in _PROGRAMS:
        _PROGRAMS[H] = build_program(H)
    return _PROGRAMS[H]


# ---------------------------------------------------------------- host side

def make_in_maps(inputs, H=64):
    W = H
    L = H * W
    LQ = L // 4
    bf = ml_dtypes.bfloat16
    order = diag_order(H)

    x = np.asarray(inputs["x"], np.float32)
    w_in = np.asarray(inputs["w_in"], np.float32)
    conv_w = np.asarray(inputs["conv_w"], np.float32)
    conv_b = np.asarray(inputs["conv_b"], np.float32)
    x_proj_w = np.asarray(inputs["x_proj_w"], np.float32)
    dt_w = np.asarray(inputs["dt_w"], np.float32)
    dt_b = np.asarray(inputs["dt_b"], np.float32)
    A_logs = np.asarray(inputs["A_logs"], np.float32)
    Ds = np.asarray(inputs["Ds"], np.float32)
    ln_g = np.asarray(inputs["ln_g"], np.float32)
    ln_b = np.asarray(inputs["ln_b"], np.float32)
    w_out = np.asarray(inputs["w_out"], np.float32)

    A_full = np.exp(A_logs).reshape(K, D, N)
    Ds2 = Ds.reshape(K, D)
    id96 = np.eye(96, dtype=np.float32)
    onc = np.ones((96, 1), np.float32)
    onr = np.ones((1, 96), np.float32)

    in_maps = []
    for c in range(NCORES):
        b, dh, nh = c // 4, (c // 2) % 2, c % 2
        lq = dh * 2 + nh
        dsl = slice(dh * 96, dh * 96 + 96)
        nsl = slice(nh * 8, nh * 8 + 8)

        xT = np.ascontiguousarray(x[b].reshape(L, DM).T)
        xTq = np.ascontiguousarray(xT[:, order[lq * LQ:(lq + 1) * LQ]])

        halves = [dsl, slice((1 - dh) * 96, (1 - dh) * 96 + 96)]
        wd = np.zeros((96, 2, 10, 96), np.float32)
        xpj = np.zeros((96, 2, 44), np.float32)
        wxi2 = np.zeros((96, 2, 96), np.float32)
        cb2 = np.zeros((96, 2), np.float32)
        for hh, hsl in enumerate(halves):
            cw = conv_w[hsl, 0].reshape(96, 9)
            for t in range(9):
                np.fill_diagonal(wd[:, hh, t, :], cw[:, t])
            np.fill_diagonal(wd[:, hh, 9, :], conv_b[hsl])
            wxi2[:, hh, :] = w_in[hsl, :].T
            cb2[:, hh] = conv_b[hsl]
            for k in range(K):
                sel = np.concatenate([
                    x_proj_w[k, 0:R, hsl],
                    -x_proj_w[k, R + nh * 8:R + nh * 8 + 8, hsl],
                    x_proj_w[k, R + N + nh * 8:R + N + nh * 8 + 8, hsl],
                ], axis=0)
                xpj[:, hh, k * 22:(k + 1) * 22] = sel.T

        dtw = np.zeros((12, 96), np.float32)
        for k in range(K):
            dtw[k * 6:(k + 1) * 6, :] = dt_w[k, dsl, :].T

        dtb = -np.stack([dt_b[0, dsl], dt_b[1, dsl]], axis=1)
        Acol = np.concatenate([A_full[0, dsl, nsl], A_full[1, dsl, nsl]], axis=1)
        dse = ((Ds2[0, dsl] + Ds2[1, dsl]) / 2.0)[:, None]
        lng = np.stack([ln_g[0:96], ln_g[96:192]], axis=1)
        lnb = np.stack([ln_b[0:96], ln_b[96:192]], axis=1)
        wz = np.ascontiguousarray(w_in[D:2 * D, :].T)
        wo = np.concatenate([w_out[:, 0:96].T, w_out[:, 96:192].T], axis=1)

        in_maps.append({
            "xT": xT.astype(bf),
            "xTq": xTq.astype(bf),
            "wxi": wxi2.reshape(96, 2 * 96).astype(bf),
            "wd": wd.reshape(96, 2 * 10 * 96).astype(bf),
            "cb": cb2.astype(np.float32),
            "xpj": xpj.reshape(96, 2 * 44).astype(bf),
            "dtw": dtw.astype(bf),
            "dtb": dtb.astype(np.float32),
            "Acol": Acol.astype(np.float32),
            "dse": dse.astype(np.float32),
            "mlo": np.full((96, 1), 1.0 if dh == 0 else 0.0, np.float32),
            "mhi": np.full((96, 1), 1.0 if dh == 1 else 0.0, np.float32),
            "id96": id96.astype(bf),
            "onc": onc.astype(bf),
            "onr": onr.astype(bf),
            "lng": lng.astype(np.float32),
            "lnb": lnb.astype(np.float32),
            "wz": wz.astype(bf),
            "wo": wo.astype(bf),
        })
    return in_maps


def assemble_output(results, H=64):
    L = H * H
    LQ = L // 4
    order = diag_order(H)
    out = np.zeros((B, L, DM), np.float32)
    for c in range(NCORES):
        b, dh, nh = c // 4, (c // 2) % 2, c % 2
        lq = dh * 2 + nh
        out[b, order[lq * LQ:(lq + 1) * LQ], :] = results[c]["out_part"].T
    return out.reshape(B, H, H, DM)


def kernel(**inputs):
    nc = get_program(64)
    in_maps = make_in_maps(inputs, 64)
    res = run_bass_kernel_spmd(nc, in_maps, core_ids=list(range(NCORES)))
    return assemble_output(res.results, 64)
